# revision 21
# baseline (speedup 1.0000x reference)
"""BEV camera-to-grid scatter kernel for Trainium2 (8 NeuronCores).

v2 design (program-size-minimal, uniform SPMD):
 - Host (O(cameras) work): compose per-camera affine geometry into per-unit
   (camera, depth, h-half) coefficients; cull units and bound each unit's BEV
   window via interval arithmetic.  The hot BEV region is tiny (~56x28 cells);
   every unit window fits a fixed-height "stripe" of the region, and stripe
   origins quantize to a handful of classes, so the PSUM->region paste offset
   is static per class.
 - Device (ONE uniform program, ~350 instructions): batched geometry + direct
   binning (f32 divide + floor) + stripe-local scatter index with penalty
   masking for all units at once; then a hardware For_i loop over units:
   stream the unit's features (fp16), build per-point one-hot rows
   (fp16 tensor_scalar is_equal vs iota), scatter-accumulate via matmuls into
   a PSUM stripe, stage to DRAM; per-class paste loops accumulate stripes
   into an SBUF-resident region at static offsets; AllReduce the tiny region.
 - Host: paste the reduced region into the (mostly zero) full output.
"""
import sys
import numpy as np

sys.path.insert(0, '/opt/trn_rl_repo')

B, N, D, FH, FW, C = 1, 6, 118, 32, 88, 80
IH, IW = 256, 704
NX, NY, NZ = 360, 360, 1
NCORES = 8
HHALF = 16
UPIX = HHALF * FW          # 1408
UJ = UPIX // 128           # 11 free columns per partition
NCOEF = 21
NMETA = 5                  # D0, kx0, kx1, ky0, ky1
BIGPEN = 60000.0
MAGIC = np.float32(2 ** 23)

_f32 = np.float32
DXV = _f32(0.3)
DZV = _f32(20.0)
# replicate the reference's f32 constant arithmetic for (bx - dx/2)
_BX = _f32(-54.0 + 0.3 / 2.0)
COFFX = _f32(_BX - DXV / _f32(2.0))
_BZ = _f32(-10.0 + 20.0 / 2.0)
COFFZ = _f32(_BZ - DZV / _f32(2.0))


def _frustum_axes():
    ds = np.arange(1.0, 60.0, 0.5, dtype=np.float32)
    xs = np.linspace(0.0, IW - 1, FW, dtype=np.float32)
    ys = np.linspace(0.0, IH - 1, FH, dtype=np.float32)
    return ds, xs, ys


def _compute_coeffs(camera2ego, lidar2ego, camera_intrinsics, img_aug_matrix, lidar_aug_matrix):
    aug = np.asarray(img_aug_matrix, np.float64)
    c2e = np.asarray(camera2ego, np.float64)
    intr = np.asarray(camera_intrinsics, np.float64)
    l2e = np.asarray(lidar2ego, np.float64)
    laug = np.asarray(lidar_aug_matrix, np.float64)
    inv_pr = np.linalg.inv(aug[..., :3, :3])
    post_trans = aug[..., :3, 3]
    A64 = inv_pr
    b64 = -np.einsum('bnij,bnj->bni', inv_pr, post_trans)
    combine = c2e[..., :3, :3] @ np.linalg.inv(intr[..., :3, :3])
    pre = laug[..., :3, :3] @ np.linalg.inv(l2e[..., :3, :3])
    M64 = np.einsum('bij,bnjk->bnik', pre, combine)
    t64 = np.einsum('bij,bnj->bni', pre, c2e[..., :3, 3] - l2e[..., :3, 3][:, None, :]) \
        + laug[..., :3, 3][:, None, :]
    return (A64[0].astype(np.float32), b64[0].astype(np.float32),
            M64[0].astype(np.float32), t64[0].astype(np.float32))


class _Iv:
    __slots__ = ('lo', 'hi')
    def __init__(self, lo, hi):
        self.lo = float(min(lo, hi)); self.hi = float(max(lo, hi))
    def __add__(self, o):
        if isinstance(o, _Iv):
            return _Iv(self.lo + o.lo, self.hi + o.hi)
        return _Iv(self.lo + o, self.hi + o)
    def __mul__(self, o):
        if isinstance(o, _Iv):
            c = [self.lo * o.lo, self.lo * o.hi, self.hi * o.lo, self.hi * o.hi]
            return _Iv(min(c), max(c))
        return _Iv(self.lo * o, self.hi * o) if o >= 0 else _Iv(self.hi * o, self.lo * o)
    __rmul__ = __mul__
    def intersect(self, o):
        lo = max(self.lo, o.lo); hi = min(self.hi, o.hi)
        return _Iv(lo, hi) if lo <= hi else None
    def pad(self, e):
        return _Iv(self.lo - e, self.hi + e)


def _cell_of(g, coff, dx):
    return int(np.floor((np.float64(g) - np.float64(coff)) / np.float64(dx)))


def _plan_units(A, b, M, t):
    ds, xs, ys = _frustum_axes()
    EPS = 2e-3
    zlo = float(COFFZ) - float(DZV) - EPS         # qz in (-1, 1)
    zhi = float(COFFZ) + float(DZV) + EPS
    units = []
    for n in range(N):
        An = A[n].astype(np.float64); bn = b[n].astype(np.float64)
        Mn = M[n].astype(np.float64); tn = t[n].astype(np.float64)
        for d in range(D):
            dv = float(ds[d])
            for half in range(FH // HHALF):
                pyv = ys[half * HHALF:(half + 1) * HHALF].astype(np.float64)
                pxI = _Iv(float(xs[0]), float(xs[-1]))
                pyI = _Iv(float(pyv[0]), float(pyv[-1]))
                p0 = [(An[i, 0] * pxI + An[i, 1] * pyI + (An[i, 2] * dv + bn[i])).pad(EPS)
                      for i in range(3)]
                zI = p0[2]
                qI = (Mn[2, 0] * p0[0] + Mn[2, 1] * p0[1] + Mn[2, 2]).pad(1e-6)
                gzI = (zI * qI + tn[2]).pad(EPS)
                if gzI.intersect(_Iv(zlo, zhi)) is None:
                    continue
                zc = zI
                if qI.lo > 1e-6 or qI.hi < -1e-6:
                    cands = [(zlo - tn[2]) / qI.lo, (zlo - tn[2]) / qI.hi,
                             (zhi - tn[2]) / qI.lo, (zhi - tn[2]) / qI.hi]
                    zc = zI.intersect(_Iv(min(cands), max(cands))) or zI
                rxI = (Mn[0, 0] * p0[0] + Mn[0, 1] * p0[1] + Mn[0, 2]).pad(1e-6)
                ryI = (Mn[1, 0] * p0[0] + Mn[1, 1] * p0[1] + Mn[1, 2]).pad(1e-6)
                gxI = (zc * rxI + tn[0]).pad(EPS)
                gyI = (zc * ryI + tn[1]).pad(EPS)
                kx0 = max(0, _cell_of(gxI.lo, COFFX, DXV) - 1)
                kx1 = min(NX - 1, _cell_of(gxI.hi, COFFX, DXV) + 1)
                ky0 = max(0, _cell_of(gyI.lo, COFFX, DXV) - 1)
                ky1 = min(NY - 1, _cell_of(gyI.hi, COFFX, DXV) + 1)
                if kx1 < kx0 or ky1 < ky0:
                    continue
                units.append(dict(n=n, d=d, half=half, kx0=kx0, wx=kx1 - kx0 + 1,
                                  ky0=ky0, wy=ky1 - ky0 + 1))
    return units


def _build_plan(inputs):
    A, b, M, t = _compute_coeffs(inputs['camera2ego'], inputs['lidar2ego'],
                                 inputs['camera_intrinsics'], inputs['img_aug_matrix'],
                                 inputs['lidar_aug_matrix'])
    units = _plan_units(A, b, M, t)
    assert units, "no units survived culling"
    rx0 = min(u['kx0'] for u in units); rx1 = max(u['kx0'] + u['wx'] for u in units)
    ry0 = min(u['ky0'] for u in units); ry1 = max(u['ky0'] + u['wy'] for u in units)
    assert rx0 > 0 and ry0 > 0, "region touches cell 0; floor!=trunc edge unsupported"
    Rx, Ry = rx1 - rx0, ry1 - ry0
    maxwy = max(u['wy'] for u in units)
    # stripe height: smallest covering height; keep W within 3 PSUM banks
    H = min(Ry, maxwy + 1)
    W = H * Rx
    assert W <= 1536, (H, Rx)
    cap = max(Ry - H, 0)
    step = max(H - maxwy + 1, 1)
    classes = list(range(0, cap + 1, step))
    if classes[-1] != cap:
        classes.append(cap)

    # assign each unit the largest class whose stripe covers its y-window
    for u in units:
        so = None
        for s in classes:
            if s <= u['ky0'] - ry0 and (u['ky0'] - ry0) + u['wy'] <= s + H:
                so = s
        assert so is not None, (u, classes, H)
        u['cls'] = so
    used = sorted({u['cls'] for u in units})
    classes = used
    cls_index = {s: i for i, s in enumerate(classes)}

    # distribute per class round-robin across cores, pad to equal counts;
    # pad the total to a multiple of 3 (main-loop unroll) with dummy slots
    percls = [[] for _ in classes]
    for i, u in enumerate(units):
        percls[cls_index[u['cls']]].append(i)
    cls_cnt2 = [-(-len(p) // NCORES) for p in percls]
    while sum(cls_cnt2) % 3:
        cls_cnt2[-1] += 1
    smax = sum(cls_cnt2)

    ds_, xs, ys = _frustum_axes()
    i = np.arange(UPIX)
    pxt = xs[i % FW].reshape(128, UJ)
    pyt_half = [ys[h * HHALF + (i // FW)].reshape(128, UJ) for h in range(FH // HHALF)]

    plan = dict(rx0=rx0, ry0=ry0, Rx=Rx, Ry=Ry, H=H, W=W, classes=classes,
                cls_cnt=cls_cnt2, smax=smax, units=units, cores=[])
    f = np.float32
    for k in range(NCORES):
        slots = []      # (slot, unit_idx or None, class_origin)
        s = 0
        for ci, cnt in enumerate(cls_cnt2):
            mine = percls[ci][k::NCORES]
            assert len(mine) <= cnt
            mine = mine + [None] * (cnt - len(mine))
            for ui in mine:
                slots.append((s, ui, classes[ci]))
                s += 1
        assert s == smax
        pyts = np.zeros((128, smax * UJ), np.float32)
        coef = np.zeros((smax, NCOEF), np.float32)
        meta = np.zeros((smax, NMETA), np.float32)
        ulist = []
        for (s, ui, so) in slots:
            if ui is not None:
                u = units[ui]
                n, d, half = u['n'], u['d'], u['half']
                dv = ds_[d]
                pyts[:, s * UJ:(s + 1) * UJ] = pyt_half[half]
                cc = []
                for kk in range(3):
                    c2 = f(f(A[n][kk, 2] * dv) + b[n][kk])
                    cc += [A[n][kk, 0], A[n][kk, 1], c2]
                for kk in range(3):
                    cc += [M[n][kk, 0], M[n][kk, 1], M[n][kk, 2], t[n][kk]]
                coef[s] = np.array(cc, np.float32)
                meta[s] = [f((so + ry0) * Rx + rx0), f(u['kx0']), f(u['kx0'] + u['wx']),
                           f(u['ky0']), f(u['ky0'] + u['wy'])]
                ulist.append(dict(slot=s, n=n, d=d, half=half, so=so))
            else:
                coef[s] = 0.0
                coef[s][20] = 1.0e9          # t_z -> gz huge -> z-pen kills all
                meta[s] = [0.0, 0.0, 0.0, 0.0, 0.0]   # kx1==kx0 -> empty window
                ulist.append(dict(slot=s, n=-1, d=-1, half=0, so=so))
        coef_t = np.broadcast_to(coef.reshape(1, smax * NCOEF), (128, smax * NCOEF)).copy()
        meta_t = np.broadcast_to(meta.reshape(1, smax * NMETA), (128, smax * NMETA)).copy()
        plan['cores'].append(dict(units=ulist, pyt=pyts, coef=coef_t, meta=meta_t))
    plan['pxt'] = np.ascontiguousarray(pxt)
    return plan


def _pack_feats(cam_feats, plan):
    """Per-core feats stack [smax, 128, UJ*C] fp16 from the culled half-slabs."""
    smax = plan['smax']
    outs = []
    cf = np.asarray(cam_feats, np.float32)[0]   # [N,D,FH,FW,C]
    for core in plan['cores']:
        fbuf = np.zeros((smax, 128, UJ * C), np.float16)
        for u in core['units']:
            if u['n'] >= 0:
                blk = cf[u['n'], u['d'], u['half'] * HHALF:(u['half'] + 1) * HHALF]
                fbuf[u['slot']] = blk.reshape(128, UJ * C)
        outs.append(fbuf)
    return outs


_CACHE = {}


def _build_bass(plan):
    import concourse.bacc as bacc
    import concourse.mybir as mybir
    import concourse.tile as tile
    from concourse.bass import ds as dslice

    smax, W, H, Rx, Ry = plan['smax'], plan['W'], plan['H'], plan['Rx'], plan['Ry']
    SJ = smax * UJ
    rcells = Rx * Ry
    f32, f16, i32 = mybir.dt.float32, mybir.dt.float16, mybir.dt.int32
    f32r = mybir.dt.float32r
    AL = mybir.AluOpType

    nc = bacc.Bacc(None, target_bir_lowering=False, num_devices=NCORES)
    feats_t = nc.dram_tensor("feats", [smax, 128, UJ * C], f16, kind="ExternalInput")
    pxt_t = nc.dram_tensor("pxt", [128, UJ], f32, kind="ExternalInput")
    pyt_t = nc.dram_tensor("pyt", [128, SJ], f32, kind="ExternalInput")
    coef_t = nc.dram_tensor("coef", [128, smax * NCOEF], f32, kind="ExternalInput")
    meta_t = nc.dram_tensor("meta", [128, smax * NMETA], f32, kind="ExternalInput")
    rout_t = nc.dram_tensor("region_out", [C, rcells], f32, kind="ExternalOutput")

    # matmul bank slices within the stripe (PSUM bank = 512 f32)
    bank_slices = [(a, min(a + 512, W)) for a in range(0, W, 512)]

    with tile.TileContext(nc) as tc:
        with tc.tile_pool(name="tabs", bufs=1) as tp, \
             tc.tile_pool(name="geo", bufs=1) as gp, \
             tc.tile_pool(name="work", bufs=3) as wp, \
             tc.tile_pool(name="oh", bufs=1) as op_, \
             tc.tile_pool(name="ps", bufs=2, space="PSUM") as pp, \
             tc.tile_pool(name="dram", bufs=1, space="DRAM") as dp:

            pxt = tp.tile([128, UJ], f32); nc.sync.dma_start(pxt[:], pxt_t[:])
            pyt = tp.tile([128, SJ], f32); nc.sync.dma_start(pyt[:], pyt_t[:])
            coef = tp.tile([128, smax * NCOEF], f32); nc.sync.dma_start(coef[:], coef_t[:])
            meta = tp.tile([128, smax * NMETA], f32); nc.sync.dma_start(meta[:], meta_t[:])
            iota32 = gp.tile([128, W], i32)
            nc.gpsimd.iota(iota32[:], pattern=[[1, W]], base=0, channel_multiplier=0)
            iotaf = tp.tile([128, W], f32)
            nc.vector.tensor_copy(out=iotaf[:], in_=iota32[:])
            region = tp.tile([C, rcells], f32)
            nc.vector.memset(region[:], 0.0)
            stag = tp.tile([C, smax * W], f16)
            lidxT = tp.tile([128, SJ], f32)

            def cslice(kidx):
                ap = coef[:].rearrange("p (s k) -> p s k", k=NCOEF)[:, :, kidx:kidx + 1]
                return ap.broadcast_to([128, smax, UJ])

            def mslice(kidx):
                ap = meta[:].rearrange("p (s k) -> p s k", k=NMETA)[:, :, kidx:kidx + 1]
                return ap.broadcast_to([128, smax, UJ])

            def g3(ap):
                return ap.rearrange("p (s j) -> p s j", j=UJ)

            # ---- batched geometry (identical op order to the reference) ----
            tmpa = gp.tile([128, SJ], f32)
            tmpb = gp.tile([128, SJ], f32)
            pxb = pxt[:][:, None, :].broadcast_to([128, smax, UJ])
            p0 = [gp.tile([128, SJ], f32, name=f'p0_{i}', tag=f'p0_{i}') for i in range(3)]
            for kk in range(3):
                nc.vector.tensor_tensor(out=g3(tmpa[:]), in0=pxb, in1=cslice(3 * kk + 0), op=AL.mult)
                nc.vector.tensor_tensor(out=g3(tmpb[:]), in0=g3(pyt[:]), in1=cslice(3 * kk + 1), op=AL.mult)
                nc.vector.tensor_tensor(out=tmpa[:], in0=tmpa[:], in1=tmpb[:], op=AL.add)
                nc.vector.tensor_tensor(out=g3(p0[kk][:]), in0=g3(tmpa[:]), in1=cslice(3 * kk + 2), op=AL.add)
            uu = gp.tile([128, SJ], f32)
            vv = gp.tile([128, SJ], f32)
            nc.vector.tensor_tensor(out=uu[:], in0=p0[0][:], in1=p0[2][:], op=AL.mult)
            nc.vector.tensor_tensor(out=vv[:], in0=p0[1][:], in1=p0[2][:], op=AL.mult)
            g = [gp.tile([128, SJ], f32, name=f'g_{i}', tag=f'g_{i}') for i in range(3)]
            for kk in range(3):
                base = 9 + 4 * kk
                nc.vector.tensor_tensor(out=g3(tmpa[:]), in0=g3(uu[:]), in1=cslice(base + 0), op=AL.mult)
                nc.vector.tensor_tensor(out=g3(tmpb[:]), in0=g3(vv[:]), in1=cslice(base + 1), op=AL.mult)
                nc.vector.tensor_tensor(out=tmpa[:], in0=tmpa[:], in1=tmpb[:], op=AL.add)
                nc.vector.tensor_tensor(out=g3(tmpb[:]), in0=g3(p0[2][:]), in1=cslice(base + 2), op=AL.mult)
                nc.vector.tensor_tensor(out=tmpa[:], in0=tmpa[:], in1=tmpb[:], op=AL.add)
                nc.vector.tensor_tensor(out=g3(g[kk][:]), in0=g3(tmpa[:]), in1=cslice(base + 3), op=AL.add)
            gx, gy, gz = g

            # ---- direct binning: q = (g - coff) / dx ; k = floor(q) ----
            qx = uu; qy = vv; qz = p0[0]        # reuse buffers
            invx = float(np.float32(1.0) / DXV)
            invz = float(np.float32(1.0) / DZV)
            nc.vector.tensor_scalar(out=qx[:], in0=gx[:], scalar1=float(COFFX),
                                    scalar2=invx, op0=AL.subtract, op1=AL.mult)
            nc.vector.tensor_scalar(out=qy[:], in0=gy[:], scalar1=float(COFFX),
                                    scalar2=invx, op0=AL.subtract, op1=AL.mult)
            nc.vector.tensor_scalar(out=qz[:], in0=gz[:], scalar1=float(COFFZ),
                                    scalar2=invz, op0=AL.subtract, op1=AL.mult)
            kxt = p0[1]; kyt = p0[2]
            # round-to-nearest then subtract (q < r) -> floor
            nc.vector.tensor_scalar(out=kxt[:], in0=qx[:], scalar1=float(MAGIC),
                                    scalar2=float(MAGIC), op0=AL.add, op1=AL.subtract)
            nc.vector.tensor_tensor(out=tmpa[:], in0=qx[:], in1=kxt[:], op=AL.is_lt)
            nc.vector.tensor_tensor(out=kxt[:], in0=kxt[:], in1=tmpa[:], op=AL.subtract)
            nc.vector.tensor_scalar(out=kyt[:], in0=qy[:], scalar1=float(MAGIC),
                                    scalar2=float(MAGIC), op0=AL.add, op1=AL.subtract)
            nc.vector.tensor_tensor(out=tmpa[:], in0=qy[:], in1=kyt[:], op=AL.is_lt)
            nc.vector.tensor_tensor(out=kyt[:], in0=kyt[:], in1=tmpa[:], op=AL.subtract)

            # ---- penalties: window containment + z in (-1, 1) ----
            pen = gx      # reuse
            nc.vector.tensor_tensor(out=g3(pen[:]), in0=g3(kxt[:]), in1=mslice(1), op=AL.is_ge)
            nc.vector.tensor_tensor(out=g3(tmpa[:]), in0=g3(kxt[:]), in1=mslice(2), op=AL.is_lt)
            nc.vector.tensor_tensor(out=pen[:], in0=pen[:], in1=tmpa[:], op=AL.mult)
            nc.vector.tensor_tensor(out=g3(tmpa[:]), in0=g3(kyt[:]), in1=mslice(3), op=AL.is_ge)
            nc.vector.tensor_tensor(out=pen[:], in0=pen[:], in1=tmpa[:], op=AL.mult)
            nc.vector.tensor_tensor(out=g3(tmpa[:]), in0=g3(kyt[:]), in1=mslice(4), op=AL.is_lt)
            nc.vector.tensor_tensor(out=pen[:], in0=pen[:], in1=tmpa[:], op=AL.mult)
            nc.vector.tensor_scalar(out=tmpa[:], in0=qz[:], scalar1=-1.0, scalar2=None, op0=AL.is_gt)
            nc.vector.tensor_tensor(out=pen[:], in0=pen[:], in1=tmpa[:], op=AL.mult)
            nc.vector.tensor_scalar(out=tmpa[:], in0=qz[:], scalar1=1.0, scalar2=None, op0=AL.is_lt)
            nc.vector.tensor_tensor(out=pen[:], in0=pen[:], in1=tmpa[:], op=AL.mult)

            # ---- stripe-local index: kyt*Rx + kxt - D0, clamp, apply penalty ----
            lidx = gy     # reuse
            nc.vector.tensor_scalar(out=lidx[:], in0=kyt[:], scalar1=float(Rx),
                                    scalar2=None, op0=AL.mult)
            nc.vector.tensor_tensor(out=lidx[:], in0=lidx[:], in1=kxt[:], op=AL.add)
            nc.vector.tensor_tensor(out=g3(lidx[:]), in0=g3(lidx[:]), in1=mslice(0), op=AL.subtract)
            nc.vector.tensor_scalar(out=lidx[:], in0=lidx[:], scalar1=-1000.0,
                                    scalar2=40000.0, op0=AL.max, op1=AL.min)
            nc.vector.tensor_scalar(out=tmpa[:], in0=pen[:], scalar1=-BIGPEN,
                                    scalar2=BIGPEN, op0=AL.mult, op1=AL.add)
            nc.vector.tensor_tensor(out=lidxT[:], in0=lidx[:], in1=tmpa[:], op=AL.add)

            region2d = region[:].rearrange("p (y x) -> p y x", x=Rx)
            DVE_JS = tuple(range(0, 5))
            POOL_JS = tuple(range(5, UJ))

            # ---- main unit loop ----
            def body(iv):
                cur = wp.tile([128, UJ], f32, tag="cur")
                nc.sync.dma_start(cur[:], lidxT[:, dslice(iv * UJ, UJ)])
                fb = wp.tile([128, UJ * C], f16, tag="fb")
                nc.sync.dma_start(fb[:], feats_t[dslice(iv, 1)].rearrange("o p x -> p (o x)"))
                oh = op_.tile([128, UJ * W], f16, tag="oh")
                for j in DVE_JS:
                    nc.vector.tensor_scalar(out=oh[:, j * W:(j + 1) * W], in0=iotaf[:],
                                            scalar1=cur[:, j:j + 1], scalar2=None,
                                            op0=AL.is_equal)
                for j in POOL_JS:
                    nc.gpsimd.tensor_scalar(out=oh[:, j * W:(j + 1) * W], in0=iotaf[:],
                                            scalar1=cur[:, j:j + 1], scalar2=None,
                                            op0=AL.is_equal)
                ps = pp.tile([C, W], f32, space="PSUM", tag="ps")
                for j in range(UJ):
                    for (a, bnd) in bank_slices:
                        nc.tensor.matmul(ps[:, a:bnd], lhsT=fb[:, j * C:(j + 1) * C],
                                         rhs=oh[:, j * W + a:j * W + bnd],
                                         start=(j == 0), stop=(j == UJ - 1))
                pss = wp.tile([C, W], f16, tag="pss")
                nc.scalar.copy(out=pss[:], in_=ps[:])
                nc.sync.dma_start(stag[:, dslice(iv * W, W)], pss[:])

            tc.For_i_unrolled(0, smax, 1, body, max_unroll=3)

            # ---- paste: per-class strided reduce over contiguous slots ----
            off = 0
            for ci, (so, cnt) in enumerate(zip(plan['classes'], plan['cls_cnt'])):
                dst = region2d[:, so:so + H, :]
                if cnt == 1:
                    src = stag[:, off * W:(off + 1) * W].rearrange("p (y x) -> p y x", x=Rx)
                    nc.vector.tensor_tensor(out=dst, in0=dst, in1=src, op=AL.add)
                else:
                    src3 = stag[:, off * W:(off + cnt) * W].rearrange(
                        "p (s w) -> p w s", w=W)
                    red = gp.tile([C, W], f32, name=f"red{ci}", tag=f"red{ci}")
                    nc.vector.tensor_reduce(out=red[:], in_=src3, axis=mybir.AxisListType.X,
                                            op=AL.add)
                    nc.vector.tensor_tensor(
                        out=dst, in0=dst,
                        in1=red[:].rearrange("p (y x) -> p y x", x=Rx), op=AL.add)
                off += cnt

            # ---- epilogue: allreduce the region ----
            rpart = dp.tile([C, rcells], f32)
            rsum = dp.tile([C, rcells], f32)
            nc.sync.dma_start(rpart[:], region[:])
            nc.gpsimd.collective_compute(
                "AllReduce", AL.add,
                replica_groups=[list(range(NCORES))],
                ins=[rpart[:]], outs=[rsum[:]])
            nc.sync.dma_start(rout_t[:], rsum[:])

    nc.compile()
    return nc


def _expected_channel_sums(plan, feats):
    """Host replica of the device binning -> per-channel sums of kept feats.

    The scatter only rearranges values, so sum_cells(region[c]) must equal
    sum_kept(feats[c]).  Used as a cheap integrity check on device output.
    """
    f32 = np.float32
    Rx = plan['Rx']
    W = plan['W']
    invx = f32(f32(1.0) / DXV)
    invz = f32(f32(1.0) / DZV)
    tot = np.zeros(C, np.float64)
    for k in range(NCORES):
        cpl = plan['cores'][k]
        smax = plan['smax']
        coef = cpl['coef'][0].reshape(smax, NCOEF)
        meta = cpl['meta'][0].reshape(smax, NMETA)
        px = np.broadcast_to(plan['pxt'][:, None, :], (128, smax, UJ)).astype(f32)
        py = cpl['pyt'].reshape(128, smax, UJ).astype(f32)
        cs = lambda i: coef[None, :, i:i + 1].astype(f32)
        p0 = []
        for kk in range(3):
            ta = f32(px * cs(3 * kk + 0)); tb = f32(py * cs(3 * kk + 1))
            p0.append(f32(f32(ta + tb) + cs(3 * kk + 2)))
        uu = f32(p0[0] * p0[2]); vv = f32(p0[1] * p0[2])
        g = []
        for kk in range(3):
            base = 9 + 4 * kk
            ta = f32(uu * cs(base + 0)); tb = f32(vv * cs(base + 1))
            ta = f32(ta + tb); tb = f32(p0[2] * cs(base + 2))
            g.append(f32(f32(ta + tb) + cs(base + 3)))
        gx, gy, gz = g
        qx = f32(f32(gx - COFFX) * invx)
        qy = f32(f32(gy - COFFX) * invx)
        qz = f32(f32(gz - COFFZ) * invz)

        def floorq(q):
            r = f32(f32(q + MAGIC) - MAGIC)
            return f32(r - (q < r).astype(f32))
        kxt = floorq(qx); kyt = floorq(qy)
        pen = ((kxt >= meta[None, :, 1:2]) & (kxt < meta[None, :, 2:3]) &
               (kyt >= meta[None, :, 3:4]) & (kyt < meta[None, :, 4:5]) &
               (qz > f32(-1.0)) & (qz < f32(1.0)))
        fb = feats[k].reshape(smax, 128, UJ, C).astype(np.float32)
        m = pen.transpose(1, 0, 2)            # [smax, 128, UJ]
        tot += np.einsum('spj,spjc->c', m.astype(np.float64), fb)
    return tot


def kernel(**inputs) -> np.ndarray:
    from concourse.bass_utils import run_bass_kernel_spmd

    plan = _build_plan(inputs)
    key = (plan['smax'], plan['W'], plan['H'], plan['Rx'], plan['Ry'],
           tuple(plan['classes']), tuple(plan['cls_cnt']))
    if key not in _CACHE:
        _CACHE.clear()
        _CACHE[key] = _build_bass(plan)

    feats = _pack_feats(inputs['cam_feats'], plan)
    in_maps = []
    for k in range(NCORES):
        cpl = plan['cores'][k]
        in_maps.append(dict(feats=feats[k], pxt=plan['pxt'], pyt=cpl['pyt'],
                            coef=cpl['coef'], meta=cpl['meta']))
    want = _expected_channel_sums(plan, feats)
    wnorm = np.linalg.norm(want) + 1.0

    region = None
    for attempt in range(4):
        r = run_bass_kernel_spmd(_CACHE[key], in_maps, core_ids=list(range(NCORES)))
        region = r.results[0]['region_out']      # [C, rcells] summed over cores
        got = region.astype(np.float64).sum(axis=1)
        if np.linalg.norm(got - want) / wnorm < 0.02:
            break
        print(f"kernel: integrity check failed (attempt {attempt}); retrying",
              flush=True)
        if attempt >= 1:
            _CACHE.clear()
            _CACHE[key] = _build_bass(plan)
    out = np.zeros((B, C, NX, NY), np.float32)
    Rx, Ry = plan['Rx'], plan['Ry']
    blk = region.reshape(C, Ry, Rx).transpose(0, 2, 1)
    out[0, :, plan['rx0']:plan['rx0'] + Rx, plan['ry0']:plan['ry0'] + Ry] = blk
    return out


# revision 23
# speedup vs baseline: 108.4791x; 108.4791x over previous
"""BEV camera-to-grid scatter kernel for Trainium2 (8 NeuronCores).

v2 design (program-size-minimal, uniform SPMD):
 - Host (O(cameras) work): compose per-camera affine geometry into per-unit
   (camera, depth, h-half) coefficients; cull units and bound each unit's BEV
   window via interval arithmetic.  The hot BEV region is tiny (~56x28 cells);
   every unit window fits a fixed-height "stripe" of the region, and stripe
   origins quantize to a handful of classes, so the PSUM->region paste offset
   is static per class.
 - Device (ONE uniform program, ~350 instructions): batched geometry + direct
   binning (f32 divide + floor) + stripe-local scatter index with penalty
   masking for all units at once; then a hardware For_i loop over units:
   stream the unit's features (fp16), build per-point one-hot rows
   (fp16 tensor_scalar is_equal vs iota), scatter-accumulate via matmuls into
   a PSUM stripe, stage to DRAM; per-class paste loops accumulate stripes
   into an SBUF-resident region at static offsets; AllReduce the tiny region.
 - Host: paste the reduced region into the (mostly zero) full output.
"""
import sys
import numpy as np

sys.path.insert(0, '/opt/trn_rl_repo')

B, N, D, FH, FW, C = 1, 6, 118, 32, 88, 80
IH, IW = 256, 704
NX, NY, NZ = 360, 360, 1
NCORES = 8
HHALF = 16
UPIX = HHALF * FW          # 1408
UJ = UPIX // 128           # 11 free columns per partition
NCOEF = 21
NMETA = 5                  # D0, kx0, kx1, ky0, ky1
BIGPEN = 60000.0
MAGIC = np.float32(2 ** 23)

_f32 = np.float32
DXV = _f32(0.3)
DZV = _f32(20.0)
# replicate the reference's f32 constant arithmetic for (bx - dx/2)
_BX = _f32(-54.0 + 0.3 / 2.0)
COFFX = _f32(_BX - DXV / _f32(2.0))
_BZ = _f32(-10.0 + 20.0 / 2.0)
COFFZ = _f32(_BZ - DZV / _f32(2.0))


def _frustum_axes():
    ds = np.arange(1.0, 60.0, 0.5, dtype=np.float32)
    xs = np.linspace(0.0, IW - 1, FW, dtype=np.float32)
    ys = np.linspace(0.0, IH - 1, FH, dtype=np.float32)
    return ds, xs, ys


def _compute_coeffs(camera2ego, lidar2ego, camera_intrinsics, img_aug_matrix, lidar_aug_matrix):
    aug = np.asarray(img_aug_matrix, np.float64)
    c2e = np.asarray(camera2ego, np.float64)
    intr = np.asarray(camera_intrinsics, np.float64)
    l2e = np.asarray(lidar2ego, np.float64)
    laug = np.asarray(lidar_aug_matrix, np.float64)
    inv_pr = np.linalg.inv(aug[..., :3, :3])
    post_trans = aug[..., :3, 3]
    A64 = inv_pr
    b64 = -np.einsum('bnij,bnj->bni', inv_pr, post_trans)
    combine = c2e[..., :3, :3] @ np.linalg.inv(intr[..., :3, :3])
    pre = laug[..., :3, :3] @ np.linalg.inv(l2e[..., :3, :3])
    M64 = np.einsum('bij,bnjk->bnik', pre, combine)
    t64 = np.einsum('bij,bnj->bni', pre, c2e[..., :3, 3] - l2e[..., :3, 3][:, None, :]) \
        + laug[..., :3, 3][:, None, :]
    return (A64[0].astype(np.float32), b64[0].astype(np.float32),
            M64[0].astype(np.float32), t64[0].astype(np.float32))


class _Iv:
    __slots__ = ('lo', 'hi')
    def __init__(self, lo, hi):
        self.lo = float(min(lo, hi)); self.hi = float(max(lo, hi))
    def __add__(self, o):
        if isinstance(o, _Iv):
            return _Iv(self.lo + o.lo, self.hi + o.hi)
        return _Iv(self.lo + o, self.hi + o)
    def __mul__(self, o):
        if isinstance(o, _Iv):
            c = [self.lo * o.lo, self.lo * o.hi, self.hi * o.lo, self.hi * o.hi]
            return _Iv(min(c), max(c))
        return _Iv(self.lo * o, self.hi * o) if o >= 0 else _Iv(self.hi * o, self.lo * o)
    __rmul__ = __mul__
    def intersect(self, o):
        lo = max(self.lo, o.lo); hi = min(self.hi, o.hi)
        return _Iv(lo, hi) if lo <= hi else None
    def pad(self, e):
        return _Iv(self.lo - e, self.hi + e)


def _cell_of(g, coff, dx):
    return int(np.floor((np.float64(g) - np.float64(coff)) / np.float64(dx)))


def _plan_units(A, b, M, t):
    ds, xs, ys = _frustum_axes()
    EPS = 2e-3
    zlo = float(COFFZ) - float(DZV) - EPS         # qz in (-1, 1)
    zhi = float(COFFZ) + float(DZV) + EPS
    units = []
    for n in range(N):
        An = A[n].astype(np.float64); bn = b[n].astype(np.float64)
        Mn = M[n].astype(np.float64); tn = t[n].astype(np.float64)
        for d in range(D):
            dv = float(ds[d])
            for half in range(FH // HHALF):
                pyv = ys[half * HHALF:(half + 1) * HHALF].astype(np.float64)
                pxI = _Iv(float(xs[0]), float(xs[-1]))
                pyI = _Iv(float(pyv[0]), float(pyv[-1]))
                p0 = [(An[i, 0] * pxI + An[i, 1] * pyI + (An[i, 2] * dv + bn[i])).pad(EPS)
                      for i in range(3)]
                zI = p0[2]
                qI = (Mn[2, 0] * p0[0] + Mn[2, 1] * p0[1] + Mn[2, 2]).pad(1e-6)
                gzI = (zI * qI + tn[2]).pad(EPS)
                if gzI.intersect(_Iv(zlo, zhi)) is None:
                    continue
                zc = zI
                if qI.lo > 1e-6 or qI.hi < -1e-6:
                    cands = [(zlo - tn[2]) / qI.lo, (zlo - tn[2]) / qI.hi,
                             (zhi - tn[2]) / qI.lo, (zhi - tn[2]) / qI.hi]
                    zc = zI.intersect(_Iv(min(cands), max(cands))) or zI
                rxI = (Mn[0, 0] * p0[0] + Mn[0, 1] * p0[1] + Mn[0, 2]).pad(1e-6)
                ryI = (Mn[1, 0] * p0[0] + Mn[1, 1] * p0[1] + Mn[1, 2]).pad(1e-6)
                gxI = (zc * rxI + tn[0]).pad(EPS)
                gyI = (zc * ryI + tn[1]).pad(EPS)
                kx0 = max(0, _cell_of(gxI.lo, COFFX, DXV) - 1)
                kx1 = min(NX - 1, _cell_of(gxI.hi, COFFX, DXV) + 1)
                ky0 = max(0, _cell_of(gyI.lo, COFFX, DXV) - 1)
                ky1 = min(NY - 1, _cell_of(gyI.hi, COFFX, DXV) + 1)
                if kx1 < kx0 or ky1 < ky0:
                    continue
                units.append(dict(n=n, d=d, half=half, kx0=kx0, wx=kx1 - kx0 + 1,
                                  ky0=ky0, wy=ky1 - ky0 + 1))
    return units


def _build_plan(inputs):
    A, b, M, t = _compute_coeffs(inputs['camera2ego'], inputs['lidar2ego'],
                                 inputs['camera_intrinsics'], inputs['img_aug_matrix'],
                                 inputs['lidar_aug_matrix'])
    units = _plan_units(A, b, M, t)
    assert units, "no units survived culling"
    rx0 = min(u['kx0'] for u in units); rx1 = max(u['kx0'] + u['wx'] for u in units)
    ry0 = min(u['ky0'] for u in units); ry1 = max(u['ky0'] + u['wy'] for u in units)
    assert rx0 > 0 and ry0 > 0, "region touches cell 0; floor!=trunc edge unsupported"
    Rx, Ry = rx1 - rx0, ry1 - ry0
    maxwy = max(u['wy'] for u in units)
    # stripe height: smallest covering height; keep W within 3 PSUM banks
    H = min(Ry, maxwy + 1)
    W = H * Rx
    assert W <= 1536, (H, Rx)
    cap = max(Ry - H, 0)
    step = max(H - maxwy + 1, 1)
    classes = list(range(0, cap + 1, step))
    if classes[-1] != cap:
        classes.append(cap)

    # assign each unit the largest class whose stripe covers its y-window
    for u in units:
        so = None
        for s in classes:
            if s <= u['ky0'] - ry0 and (u['ky0'] - ry0) + u['wy'] <= s + H:
                so = s
        assert so is not None, (u, classes, H)
        u['cls'] = so
    used = sorted({u['cls'] for u in units})
    classes = used
    cls_index = {s: i for i, s in enumerate(classes)}

    # distribute per class round-robin across cores, pad to equal counts;
    # pad the total to a multiple of 3 (main-loop unroll) with dummy slots
    percls = [[] for _ in classes]
    for i, u in enumerate(units):
        percls[cls_index[u['cls']]].append(i)
    cls_cnt2 = [-(-len(p) // NCORES) for p in percls]
    while sum(cls_cnt2) % 3:
        cls_cnt2[-1] += 1
    smax = sum(cls_cnt2)

    ds_, xs, ys = _frustum_axes()
    i = np.arange(UPIX)
    pxt = xs[i % FW].reshape(128, UJ)
    pyt_half = [ys[h * HHALF + (i // FW)].reshape(128, UJ) for h in range(FH // HHALF)]

    plan = dict(rx0=rx0, ry0=ry0, Rx=Rx, Ry=Ry, H=H, W=W, classes=classes,
                cls_cnt=cls_cnt2, smax=smax, units=units, cores=[])
    f = np.float32
    for k in range(NCORES):
        slots = []      # (slot, unit_idx or None, class_origin)
        s = 0
        for ci, cnt in enumerate(cls_cnt2):
            mine = percls[ci][k::NCORES]
            assert len(mine) <= cnt
            mine = mine + [None] * (cnt - len(mine))
            for ui in mine:
                slots.append((s, ui, classes[ci]))
                s += 1
        assert s == smax
        pyts = np.zeros((128, smax * UJ), np.float32)
        coef = np.zeros((smax, NCOEF), np.float32)
        meta = np.zeros((smax, NMETA), np.float32)
        ulist = []
        for (s, ui, so) in slots:
            if ui is not None:
                u = units[ui]
                n, d, half = u['n'], u['d'], u['half']
                dv = ds_[d]
                pyts[:, s * UJ:(s + 1) * UJ] = pyt_half[half]
                cc = []
                for kk in range(3):
                    c2 = f(f(A[n][kk, 2] * dv) + b[n][kk])
                    cc += [A[n][kk, 0], A[n][kk, 1], c2]
                for kk in range(3):
                    cc += [M[n][kk, 0], M[n][kk, 1], M[n][kk, 2], t[n][kk]]
                coef[s] = np.array(cc, np.float32)
                meta[s] = [f((so + ry0) * Rx + rx0), f(u['kx0']), f(u['kx0'] + u['wx']),
                           f(u['ky0']), f(u['ky0'] + u['wy'])]
                ulist.append(dict(slot=s, n=n, d=d, half=half, so=so))
            else:
                coef[s] = 0.0
                coef[s][20] = 1.0e9          # t_z -> gz huge -> z-pen kills all
                meta[s] = [0.0, 0.0, 0.0, 0.0, 0.0]   # kx1==kx0 -> empty window
                ulist.append(dict(slot=s, n=-1, d=-1, half=0, so=so))
        coef_t = np.broadcast_to(coef.reshape(1, smax * NCOEF), (128, smax * NCOEF)).copy()
        meta_t = np.broadcast_to(meta.reshape(1, smax * NMETA), (128, smax * NMETA)).copy()
        plan['cores'].append(dict(units=ulist, pyt=pyts, coef=coef_t, meta=meta_t))
    plan['pxt'] = np.ascontiguousarray(pxt)
    return plan


def _pack_feats(cam_feats, plan):
    """Per-core feats stack [smax, 128, UJ*C] fp16 from the culled half-slabs."""
    smax = plan['smax']
    outs = []
    cf = np.asarray(cam_feats, np.float32)[0]   # [N,D,FH,FW,C]
    for core in plan['cores']:
        fbuf = np.zeros((smax, 128, UJ * C), np.float16)
        for u in core['units']:
            if u['n'] >= 0:
                blk = cf[u['n'], u['d'], u['half'] * HHALF:(u['half'] + 1) * HHALF]
                fbuf[u['slot']] = blk.reshape(128, UJ * C)
        outs.append(fbuf)
    return outs


_CACHE = {}


def _build_bass(plan):
    import concourse.bacc as bacc
    import concourse.mybir as mybir
    import concourse.tile as tile
    from concourse.bass import ds as dslice

    smax, W, H, Rx, Ry = plan['smax'], plan['W'], plan['H'], plan['Rx'], plan['Ry']
    SJ = smax * UJ
    rcells = Rx * Ry
    f32, f16, i32 = mybir.dt.float32, mybir.dt.float16, mybir.dt.int32
    f32r = mybir.dt.float32r
    AL = mybir.AluOpType

    nc = bacc.Bacc(None, target_bir_lowering=False, num_devices=NCORES)
    feats_t = nc.dram_tensor("feats", [smax, 128, UJ * C], f16, kind="ExternalInput")
    pxt_t = nc.dram_tensor("pxt", [128, UJ], f32, kind="ExternalInput")
    pyt_t = nc.dram_tensor("pyt", [128, SJ], f32, kind="ExternalInput")
    coef_t = nc.dram_tensor("coef", [128, smax * NCOEF], f32, kind="ExternalInput")
    meta_t = nc.dram_tensor("meta", [128, smax * NMETA], f32, kind="ExternalInput")
    rout_t = nc.dram_tensor("region_out", [C, rcells], f32, kind="ExternalOutput")

    # matmul bank slices within the stripe (PSUM bank = 512 f32)
    bank_slices = [(a, min(a + 512, W)) for a in range(0, W, 512)]

    with tile.TileContext(nc) as tc:
        with tc.tile_pool(name="tabs", bufs=1) as tp, \
             tc.tile_pool(name="geo", bufs=1) as gp, \
             tc.tile_pool(name="work", bufs=3) as wp, \
             tc.tile_pool(name="oh", bufs=1) as op_, \
             tc.tile_pool(name="ps", bufs=2, space="PSUM") as pp, \
             tc.tile_pool(name="dram", bufs=1, space="DRAM") as dp:

            pxt = tp.tile([128, UJ], f32); nc.sync.dma_start(pxt[:], pxt_t[:])
            pyt = tp.tile([128, SJ], f32); nc.sync.dma_start(pyt[:], pyt_t[:])
            coef = tp.tile([128, smax * NCOEF], f32); nc.sync.dma_start(coef[:], coef_t[:])
            meta = tp.tile([128, smax * NMETA], f32); nc.sync.dma_start(meta[:], meta_t[:])
            iota32 = gp.tile([128, W], i32)
            nc.gpsimd.iota(iota32[:], pattern=[[1, W]], base=0, channel_multiplier=0)
            iotaf = tp.tile([128, W], f32)
            nc.vector.tensor_copy(out=iotaf[:], in_=iota32[:])
            region = tp.tile([C, rcells], f32)
            nc.vector.memset(region[:], 0.0)
            stag = tp.tile([C, smax * W], f16)
            lidxT = tp.tile([128, SJ], f32)

            def cslice(kidx):
                ap = coef[:].rearrange("p (s k) -> p s k", k=NCOEF)[:, :, kidx:kidx + 1]
                return ap.broadcast_to([128, smax, UJ])

            def mslice(kidx):
                ap = meta[:].rearrange("p (s k) -> p s k", k=NMETA)[:, :, kidx:kidx + 1]
                return ap.broadcast_to([128, smax, UJ])

            def g3(ap):
                return ap.rearrange("p (s j) -> p s j", j=UJ)

            # ---- batched geometry (identical op order to the reference) ----
            tmpa = gp.tile([128, SJ], f32)
            tmpb = gp.tile([128, SJ], f32)
            pxb = pxt[:][:, None, :].broadcast_to([128, smax, UJ])
            p0 = [gp.tile([128, SJ], f32, name=f'p0_{i}', tag=f'p0_{i}') for i in range(3)]
            for kk in range(3):
                nc.vector.tensor_tensor(out=g3(tmpa[:]), in0=pxb, in1=cslice(3 * kk + 0), op=AL.mult)
                nc.vector.tensor_tensor(out=g3(tmpb[:]), in0=g3(pyt[:]), in1=cslice(3 * kk + 1), op=AL.mult)
                nc.vector.tensor_tensor(out=tmpa[:], in0=tmpa[:], in1=tmpb[:], op=AL.add)
                nc.vector.tensor_tensor(out=g3(p0[kk][:]), in0=g3(tmpa[:]), in1=cslice(3 * kk + 2), op=AL.add)
            uu = gp.tile([128, SJ], f32)
            vv = gp.tile([128, SJ], f32)
            nc.vector.tensor_tensor(out=uu[:], in0=p0[0][:], in1=p0[2][:], op=AL.mult)
            nc.vector.tensor_tensor(out=vv[:], in0=p0[1][:], in1=p0[2][:], op=AL.mult)
            g = [gp.tile([128, SJ], f32, name=f'g_{i}', tag=f'g_{i}') for i in range(3)]
            for kk in range(3):
                base = 9 + 4 * kk
                nc.vector.tensor_tensor(out=g3(tmpa[:]), in0=g3(uu[:]), in1=cslice(base + 0), op=AL.mult)
                nc.vector.tensor_tensor(out=g3(tmpb[:]), in0=g3(vv[:]), in1=cslice(base + 1), op=AL.mult)
                nc.vector.tensor_tensor(out=tmpa[:], in0=tmpa[:], in1=tmpb[:], op=AL.add)
                nc.vector.tensor_tensor(out=g3(tmpb[:]), in0=g3(p0[2][:]), in1=cslice(base + 2), op=AL.mult)
                nc.vector.tensor_tensor(out=tmpa[:], in0=tmpa[:], in1=tmpb[:], op=AL.add)
                nc.vector.tensor_tensor(out=g3(g[kk][:]), in0=g3(tmpa[:]), in1=cslice(base + 3), op=AL.add)
            gx, gy, gz = g

            # ---- direct binning: q = (g - coff) / dx ; k = floor(q) ----
            qx = uu; qy = vv; qz = p0[0]        # reuse buffers
            invx = float(np.float32(1.0) / DXV)
            invz = float(np.float32(1.0) / DZV)
            nc.vector.tensor_scalar(out=qx[:], in0=gx[:], scalar1=float(COFFX),
                                    scalar2=invx, op0=AL.subtract, op1=AL.mult)
            nc.vector.tensor_scalar(out=qy[:], in0=gy[:], scalar1=float(COFFX),
                                    scalar2=invx, op0=AL.subtract, op1=AL.mult)
            nc.vector.tensor_scalar(out=qz[:], in0=gz[:], scalar1=float(COFFZ),
                                    scalar2=invz, op0=AL.subtract, op1=AL.mult)
            kxt = p0[1]; kyt = p0[2]
            # round-to-nearest then subtract (q < r) -> floor
            nc.vector.tensor_scalar(out=kxt[:], in0=qx[:], scalar1=float(MAGIC),
                                    scalar2=float(MAGIC), op0=AL.add, op1=AL.subtract)
            nc.vector.tensor_tensor(out=tmpa[:], in0=qx[:], in1=kxt[:], op=AL.is_lt)
            nc.vector.tensor_tensor(out=kxt[:], in0=kxt[:], in1=tmpa[:], op=AL.subtract)
            nc.vector.tensor_scalar(out=kyt[:], in0=qy[:], scalar1=float(MAGIC),
                                    scalar2=float(MAGIC), op0=AL.add, op1=AL.subtract)
            nc.vector.tensor_tensor(out=tmpa[:], in0=qy[:], in1=kyt[:], op=AL.is_lt)
            nc.vector.tensor_tensor(out=kyt[:], in0=kyt[:], in1=tmpa[:], op=AL.subtract)

            # ---- penalties: window containment + z in (-1, 1) ----
            pen = gx      # reuse
            nc.vector.tensor_tensor(out=g3(pen[:]), in0=g3(kxt[:]), in1=mslice(1), op=AL.is_ge)
            nc.vector.tensor_tensor(out=g3(tmpa[:]), in0=g3(kxt[:]), in1=mslice(2), op=AL.is_lt)
            nc.vector.tensor_tensor(out=pen[:], in0=pen[:], in1=tmpa[:], op=AL.mult)
            nc.vector.tensor_tensor(out=g3(tmpa[:]), in0=g3(kyt[:]), in1=mslice(3), op=AL.is_ge)
            nc.vector.tensor_tensor(out=pen[:], in0=pen[:], in1=tmpa[:], op=AL.mult)
            nc.vector.tensor_tensor(out=g3(tmpa[:]), in0=g3(kyt[:]), in1=mslice(4), op=AL.is_lt)
            nc.vector.tensor_tensor(out=pen[:], in0=pen[:], in1=tmpa[:], op=AL.mult)
            nc.vector.tensor_scalar(out=tmpa[:], in0=qz[:], scalar1=-1.0, scalar2=None, op0=AL.is_gt)
            nc.vector.tensor_tensor(out=pen[:], in0=pen[:], in1=tmpa[:], op=AL.mult)
            nc.vector.tensor_scalar(out=tmpa[:], in0=qz[:], scalar1=1.0, scalar2=None, op0=AL.is_lt)
            nc.vector.tensor_tensor(out=pen[:], in0=pen[:], in1=tmpa[:], op=AL.mult)

            # ---- stripe-local index: kyt*Rx + kxt - D0, clamp, apply penalty ----
            lidx = gy     # reuse
            nc.vector.tensor_scalar(out=lidx[:], in0=kyt[:], scalar1=float(Rx),
                                    scalar2=None, op0=AL.mult)
            nc.vector.tensor_tensor(out=lidx[:], in0=lidx[:], in1=kxt[:], op=AL.add)
            nc.vector.tensor_tensor(out=g3(lidx[:]), in0=g3(lidx[:]), in1=mslice(0), op=AL.subtract)
            nc.vector.tensor_scalar(out=lidx[:], in0=lidx[:], scalar1=-1000.0,
                                    scalar2=40000.0, op0=AL.max, op1=AL.min)
            nc.vector.tensor_scalar(out=tmpa[:], in0=pen[:], scalar1=-BIGPEN,
                                    scalar2=BIGPEN, op0=AL.mult, op1=AL.add)
            nc.vector.tensor_tensor(out=lidxT[:], in0=lidx[:], in1=tmpa[:], op=AL.add)

            region2d = region[:].rearrange("p (y x) -> p y x", x=Rx)
            DVE_JS = tuple(range(0, 5))
            POOL_JS = tuple(range(5, UJ))

            # ---- main unit loop ----
            def body(iv):
                cur = wp.tile([128, UJ], f32, tag="cur")
                nc.sync.dma_start(cur[:], lidxT[:, dslice(iv * UJ, UJ)])
                fb = wp.tile([128, UJ * C], f16, tag="fb")
                nc.sync.dma_start(fb[:], feats_t[dslice(iv, 1)].rearrange("o p x -> p (o x)"))
                oh = op_.tile([128, UJ * W], f16, tag="oh")
                for j in DVE_JS:
                    nc.vector.tensor_scalar(out=oh[:, j * W:(j + 1) * W], in0=iotaf[:],
                                            scalar1=cur[:, j:j + 1], scalar2=None,
                                            op0=AL.is_equal)
                for j in POOL_JS:
                    nc.gpsimd.tensor_scalar(out=oh[:, j * W:(j + 1) * W], in0=iotaf[:],
                                            scalar1=cur[:, j:j + 1], scalar2=None,
                                            op0=AL.is_equal)
                ps = pp.tile([C, W], f32, space="PSUM", tag="ps")
                for j in range(UJ):
                    for (a, bnd) in bank_slices:
                        nc.tensor.matmul(ps[:, a:bnd], lhsT=fb[:, j * C:(j + 1) * C],
                                         rhs=oh[:, j * W + a:j * W + bnd],
                                         start=(j == 0), stop=(j == UJ - 1))
                pss = wp.tile([C, W], f16, tag="pss")
                nc.scalar.copy(out=pss[:], in_=ps[:])
                nc.sync.dma_start(stag[:, dslice(iv * W, W)], pss[:])

            tc.For_i_unrolled(0, smax, 1, body, max_unroll=3)

            # ---- paste: per-class strided reduce over contiguous slots ----
            off = 0
            for ci, (so, cnt) in enumerate(zip(plan['classes'], plan['cls_cnt'])):
                dst = region2d[:, so:so + H, :]
                if cnt == 1:
                    src = stag[:, off * W:(off + 1) * W].rearrange("p (y x) -> p y x", x=Rx)
                    nc.vector.tensor_tensor(out=dst, in0=dst, in1=src, op=AL.add)
                else:
                    src3 = stag[:, off * W:(off + cnt) * W].rearrange(
                        "p (s w) -> p w s", w=W)
                    red = gp.tile([C, W], f32, name=f"red{ci}", tag=f"red{ci}")
                    nc.vector.tensor_reduce(out=red[:], in_=src3, axis=mybir.AxisListType.X,
                                            op=AL.add)
                    nc.vector.tensor_tensor(
                        out=dst, in0=dst,
                        in1=red[:].rearrange("p (y x) -> p y x", x=Rx), op=AL.add)
                off += cnt

            # ---- epilogue: allreduce the region ----
            rpart = dp.tile([C, rcells], f32)
            rsum = dp.tile([C, rcells], f32)
            nc.sync.dma_start(rpart[:], region[:])
            nc.gpsimd.collective_compute(
                "AllReduce", AL.add,
                replica_groups=[list(range(NCORES))],
                ins=[rpart[:]], outs=[rsum[:]])
            nc.sync.dma_start(rout_t[:], rsum[:])

    nc.compile()
    return nc


def _expected_channel_sums(plan, feats):
    """Host replica of the device binning -> per-channel sums of kept feats.

    The scatter only rearranges values, so sum_cells(region[c]) must equal
    sum_kept(feats[c]).  Used as a cheap integrity check on device output.
    """
    f32 = np.float32
    Rx = plan['Rx']
    W = plan['W']
    invx = f32(f32(1.0) / DXV)
    invz = f32(f32(1.0) / DZV)
    tot = np.zeros(C, np.float64)
    for k in range(NCORES):
        cpl = plan['cores'][k]
        smax = plan['smax']
        coef = cpl['coef'][0].reshape(smax, NCOEF)
        meta = cpl['meta'][0].reshape(smax, NMETA)
        px = np.broadcast_to(plan['pxt'][:, None, :], (128, smax, UJ)).astype(f32)
        py = cpl['pyt'].reshape(128, smax, UJ).astype(f32)
        cs = lambda i: coef[None, :, i:i + 1].astype(f32)
        p0 = []
        for kk in range(3):
            ta = f32(px * cs(3 * kk + 0)); tb = f32(py * cs(3 * kk + 1))
            p0.append(f32(f32(ta + tb) + cs(3 * kk + 2)))
        uu = f32(p0[0] * p0[2]); vv = f32(p0[1] * p0[2])
        g = []
        for kk in range(3):
            base = 9 + 4 * kk
            ta = f32(uu * cs(base + 0)); tb = f32(vv * cs(base + 1))
            ta = f32(ta + tb); tb = f32(p0[2] * cs(base + 2))
            g.append(f32(f32(ta + tb) + cs(base + 3)))
        gx, gy, gz = g
        qx = f32(f32(gx - COFFX) * invx)
        qy = f32(f32(gy - COFFX) * invx)
        qz = f32(f32(gz - COFFZ) * invz)

        def floorq(q):
            r = f32(f32(q + MAGIC) - MAGIC)
            return f32(r - (q < r).astype(f32))
        kxt = floorq(qx); kyt = floorq(qy)
        pen = ((kxt >= meta[None, :, 1:2]) & (kxt < meta[None, :, 2:3]) &
               (kyt >= meta[None, :, 3:4]) & (kyt < meta[None, :, 4:5]) &
               (qz > f32(-1.0)) & (qz < f32(1.0)))
        fb = feats[k].reshape(smax, 128, UJ, C).astype(np.float32)
        m = pen.transpose(1, 0, 2)            # [smax, 128, UJ]
        tot += np.einsum('spj,spjc->c', m.astype(np.float64), fb)
    return tot


def kernel(**inputs) -> np.ndarray:
    from concourse.bass_utils import run_bass_kernel_spmd

    plan = _build_plan(inputs)
    key = (plan['smax'], plan['W'], plan['H'], plan['Rx'], plan['Ry'],
           tuple(plan['classes']), tuple(plan['cls_cnt']))
    if key not in _CACHE:
        _CACHE.clear()
        _CACHE[key] = _build_bass(plan)

    feats = _pack_feats(inputs['cam_feats'], plan)
    in_maps = []
    for k in range(NCORES):
        cpl = plan['cores'][k]
        in_maps.append(dict(feats=feats[k], pxt=plan['pxt'], pyt=cpl['pyt'],
                            coef=cpl['coef'], meta=cpl['meta']))
    want = _expected_channel_sums(plan, feats)
    wnorm = np.linalg.norm(want) + 1.0

    region = None
    last_exc = None
    for attempt in range(5):
        try:
            r = run_bass_kernel_spmd(_CACHE[key], in_maps, core_ids=list(range(NCORES)))
            cand = r.results[0]['region_out']    # [C, rcells] summed over cores
            got = cand.astype(np.float64).sum(axis=1)
            region = cand
            if np.linalg.norm(got - want) / wnorm < 0.02:
                break
            print(f"kernel: integrity check failed (attempt {attempt}); retrying",
                  flush=True)
        except Exception as e:
            last_exc = e
            print(f"kernel: execute failed ({type(e).__name__}); retrying", flush=True)
            import time as _time
            _time.sleep(5.0)
        if attempt >= 1:
            _CACHE.clear()
            _CACHE[key] = _build_bass(plan)
    if region is None:
        raise last_exc
    out = np.zeros((B, C, NX, NY), np.float32)
    Rx, Ry = plan['Rx'], plan['Ry']
    blk = region.reshape(C, Ry, Rx).transpose(0, 2, 1)
    out[0, :, plan['rx0']:plan['rx0'] + Rx, plan['ry0']:plan['ry0'] + Ry] = blk
    return out


# revision 25
# speedup vs baseline: 1019.7910x; 9.4008x over previous
"""BEV camera-to-grid scatter kernel for Trainium2 (8 NeuronCores).

v2 design (program-size-minimal, uniform SPMD):
 - Host (O(cameras) work): compose per-camera affine geometry into per-unit
   (camera, depth, h-half) coefficients; cull units and bound each unit's BEV
   window via interval arithmetic.  The hot BEV region is tiny (~56x28 cells);
   every unit window fits a fixed-height "stripe" of the region, and stripe
   origins quantize to a handful of classes, so the PSUM->region paste offset
   is static per class.
 - Device (ONE uniform program, ~350 instructions): batched geometry + direct
   binning (f32 divide + floor) + stripe-local scatter index with penalty
   masking for all units at once; then a hardware For_i loop over units:
   stream the unit's features (fp16), build per-point one-hot rows
   (fp16 tensor_scalar is_equal vs iota), scatter-accumulate via matmuls into
   a PSUM stripe, stage to DRAM; per-class paste loops accumulate stripes
   into an SBUF-resident region at static offsets; AllReduce the tiny region.
 - Host: paste the reduced region into the (mostly zero) full output.
"""
import sys
import numpy as np

sys.path.insert(0, '/opt/trn_rl_repo')

B, N, D, FH, FW, C = 1, 6, 118, 32, 88, 80
IH, IW = 256, 704
NX, NY, NZ = 360, 360, 1
NCORES = 8
HHALF = 16
UPIX = HHALF * FW          # 1408
UJ = UPIX // 128           # 11 free columns per partition
NCOEF = 21
NMETA = 5                  # D0, kx0, kx1, ky0, ky1
BIGPEN = 60000.0
MAGIC = np.float32(2 ** 23)

_f32 = np.float32
DXV = _f32(0.3)
DZV = _f32(20.0)
# replicate the reference's f32 constant arithmetic for (bx - dx/2)
_BX = _f32(-54.0 + 0.3 / 2.0)
COFFX = _f32(_BX - DXV / _f32(2.0))
_BZ = _f32(-10.0 + 20.0 / 2.0)
COFFZ = _f32(_BZ - DZV / _f32(2.0))


def _frustum_axes():
    ds = np.arange(1.0, 60.0, 0.5, dtype=np.float32)
    xs = np.linspace(0.0, IW - 1, FW, dtype=np.float32)
    ys = np.linspace(0.0, IH - 1, FH, dtype=np.float32)
    return ds, xs, ys


def _compute_coeffs(camera2ego, lidar2ego, camera_intrinsics, img_aug_matrix, lidar_aug_matrix):
    aug = np.asarray(img_aug_matrix, np.float64)
    c2e = np.asarray(camera2ego, np.float64)
    intr = np.asarray(camera_intrinsics, np.float64)
    l2e = np.asarray(lidar2ego, np.float64)
    laug = np.asarray(lidar_aug_matrix, np.float64)
    inv_pr = np.linalg.inv(aug[..., :3, :3])
    post_trans = aug[..., :3, 3]
    A64 = inv_pr
    b64 = -np.einsum('bnij,bnj->bni', inv_pr, post_trans)
    combine = c2e[..., :3, :3] @ np.linalg.inv(intr[..., :3, :3])
    pre = laug[..., :3, :3] @ np.linalg.inv(l2e[..., :3, :3])
    M64 = np.einsum('bij,bnjk->bnik', pre, combine)
    t64 = np.einsum('bij,bnj->bni', pre, c2e[..., :3, 3] - l2e[..., :3, 3][:, None, :]) \
        + laug[..., :3, 3][:, None, :]
    return (A64[0].astype(np.float32), b64[0].astype(np.float32),
            M64[0].astype(np.float32), t64[0].astype(np.float32))


class _Iv:
    __slots__ = ('lo', 'hi')
    def __init__(self, lo, hi):
        self.lo = float(min(lo, hi)); self.hi = float(max(lo, hi))
    def __add__(self, o):
        if isinstance(o, _Iv):
            return _Iv(self.lo + o.lo, self.hi + o.hi)
        return _Iv(self.lo + o, self.hi + o)
    def __mul__(self, o):
        if isinstance(o, _Iv):
            c = [self.lo * o.lo, self.lo * o.hi, self.hi * o.lo, self.hi * o.hi]
            return _Iv(min(c), max(c))
        return _Iv(self.lo * o, self.hi * o) if o >= 0 else _Iv(self.hi * o, self.lo * o)
    __rmul__ = __mul__
    def intersect(self, o):
        lo = max(self.lo, o.lo); hi = min(self.hi, o.hi)
        return _Iv(lo, hi) if lo <= hi else None
    def pad(self, e):
        return _Iv(self.lo - e, self.hi + e)


def _cell_of(g, coff, dx):
    return int(np.floor((np.float64(g) - np.float64(coff)) / np.float64(dx)))


def _plan_units(A, b, M, t):
    ds, xs, ys = _frustum_axes()
    EPS = 2e-3
    zlo = float(COFFZ) - float(DZV) - EPS         # qz in (-1, 1)
    zhi = float(COFFZ) + float(DZV) + EPS
    units = []
    for n in range(N):
        An = A[n].astype(np.float64); bn = b[n].astype(np.float64)
        Mn = M[n].astype(np.float64); tn = t[n].astype(np.float64)
        for d in range(D):
            dv = float(ds[d])
            for half in range(FH // HHALF):
                pyv = ys[half * HHALF:(half + 1) * HHALF].astype(np.float64)
                pxI = _Iv(float(xs[0]), float(xs[-1]))
                pyI = _Iv(float(pyv[0]), float(pyv[-1]))
                p0 = [(An[i, 0] * pxI + An[i, 1] * pyI + (An[i, 2] * dv + bn[i])).pad(EPS)
                      for i in range(3)]
                zI = p0[2]
                qI = (Mn[2, 0] * p0[0] + Mn[2, 1] * p0[1] + Mn[2, 2]).pad(1e-6)
                gzI = (zI * qI + tn[2]).pad(EPS)
                if gzI.intersect(_Iv(zlo, zhi)) is None:
                    continue
                zc = zI
                if qI.lo > 1e-6 or qI.hi < -1e-6:
                    cands = [(zlo - tn[2]) / qI.lo, (zlo - tn[2]) / qI.hi,
                             (zhi - tn[2]) / qI.lo, (zhi - tn[2]) / qI.hi]
                    zc = zI.intersect(_Iv(min(cands), max(cands))) or zI
                rxI = (Mn[0, 0] * p0[0] + Mn[0, 1] * p0[1] + Mn[0, 2]).pad(1e-6)
                ryI = (Mn[1, 0] * p0[0] + Mn[1, 1] * p0[1] + Mn[1, 2]).pad(1e-6)
                gxI = (zc * rxI + tn[0]).pad(EPS)
                gyI = (zc * ryI + tn[1]).pad(EPS)
                kx0 = max(0, _cell_of(gxI.lo, COFFX, DXV) - 1)
                kx1 = min(NX - 1, _cell_of(gxI.hi, COFFX, DXV) + 1)
                ky0 = max(0, _cell_of(gyI.lo, COFFX, DXV) - 1)
                ky1 = min(NY - 1, _cell_of(gyI.hi, COFFX, DXV) + 1)
                if kx1 < kx0 or ky1 < ky0:
                    continue
                units.append(dict(n=n, d=d, half=half, kx0=kx0, wx=kx1 - kx0 + 1,
                                  ky0=ky0, wy=ky1 - ky0 + 1))
    return units


def _build_plan(inputs):
    A, b, M, t = _compute_coeffs(inputs['camera2ego'], inputs['lidar2ego'],
                                 inputs['camera_intrinsics'], inputs['img_aug_matrix'],
                                 inputs['lidar_aug_matrix'])
    units = _plan_units(A, b, M, t)
    assert units, "no units survived culling"
    rx0 = min(u['kx0'] for u in units); rx1 = max(u['kx0'] + u['wx'] for u in units)
    ry0 = min(u['ky0'] for u in units); ry1 = max(u['ky0'] + u['wy'] for u in units)
    assert rx0 > 0 and ry0 > 0, "region touches cell 0; floor!=trunc edge unsupported"
    Rx, Ry = rx1 - rx0, ry1 - ry0
    maxwy = max(u['wy'] for u in units)
    # stripe height: smallest covering height; keep W within 3 PSUM banks
    H = min(Ry, maxwy + 1)
    W = H * Rx
    assert W <= 1536, (H, Rx)
    cap = max(Ry - H, 0)
    step = max(H - maxwy + 1, 1)
    classes = list(range(0, cap + 1, step))
    if classes[-1] != cap:
        classes.append(cap)

    # assign each unit the largest class whose stripe covers its y-window
    for u in units:
        so = None
        for s in classes:
            if s <= u['ky0'] - ry0 and (u['ky0'] - ry0) + u['wy'] <= s + H:
                so = s
        assert so is not None, (u, classes, H)
        u['cls'] = so
    used = sorted({u['cls'] for u in units})
    classes = used
    cls_index = {s: i for i, s in enumerate(classes)}

    # distribute per class round-robin across cores, pad to equal counts;
    # pad the total to a multiple of 3 (main-loop unroll) with dummy slots
    percls = [[] for _ in classes]
    for i, u in enumerate(units):
        percls[cls_index[u['cls']]].append(i)
    cls_cnt2 = [-(-len(p) // NCORES) for p in percls]
    while sum(cls_cnt2) % 3:
        cls_cnt2[-1] += 1
    smax = sum(cls_cnt2)

    ds_, xs, ys = _frustum_axes()
    i = np.arange(UPIX)
    pxt = xs[i % FW].reshape(128, UJ)
    pyt_half = [ys[h * HHALF + (i // FW)].reshape(128, UJ) for h in range(FH // HHALF)]

    plan = dict(rx0=rx0, ry0=ry0, Rx=Rx, Ry=Ry, H=H, W=W, classes=classes,
                cls_cnt=cls_cnt2, smax=smax, units=units, cores=[])
    f = np.float32
    for k in range(NCORES):
        slots = []      # (slot, unit_idx or None, class_origin)
        s = 0
        for ci, cnt in enumerate(cls_cnt2):
            mine = percls[ci][k::NCORES]
            assert len(mine) <= cnt
            mine = mine + [None] * (cnt - len(mine))
            for ui in mine:
                slots.append((s, ui, classes[ci]))
                s += 1
        assert s == smax
        pyts = np.zeros((128, smax * UJ), np.float32)
        coef = np.zeros((smax, NCOEF), np.float32)
        meta = np.zeros((smax, NMETA), np.float32)
        ulist = []
        for (s, ui, so) in slots:
            if ui is not None:
                u = units[ui]
                n, d, half = u['n'], u['d'], u['half']
                dv = ds_[d]
                pyts[:, s * UJ:(s + 1) * UJ] = pyt_half[half]
                cc = []
                for kk in range(3):
                    c2 = f(f(A[n][kk, 2] * dv) + b[n][kk])
                    cc += [A[n][kk, 0], A[n][kk, 1], c2]
                for kk in range(3):
                    cc += [M[n][kk, 0], M[n][kk, 1], M[n][kk, 2], t[n][kk]]
                coef[s] = np.array(cc, np.float32)
                meta[s] = [f((so + ry0) * Rx + rx0), f(u['kx0']), f(u['kx0'] + u['wx']),
                           f(u['ky0']), f(u['ky0'] + u['wy'])]
                ulist.append(dict(slot=s, n=n, d=d, half=half, so=so))
            else:
                coef[s] = 0.0
                coef[s][20] = 1.0e9          # t_z -> gz huge -> z-pen kills all
                meta[s] = [0.0, 0.0, 0.0, 0.0, 0.0]   # kx1==kx0 -> empty window
                ulist.append(dict(slot=s, n=-1, d=-1, half=0, so=so))
        coef_t = np.broadcast_to(coef.reshape(1, smax * NCOEF), (128, smax * NCOEF)).copy()
        meta_t = np.broadcast_to(meta.reshape(1, smax * NMETA), (128, smax * NMETA)).copy()
        plan['cores'].append(dict(units=ulist, pyt=pyts, coef=coef_t, meta=meta_t))
    plan['pxt'] = np.ascontiguousarray(pxt)
    return plan


def _pack_feats(cam_feats, plan):
    """Per-core feats stack [smax, 128, UJ*C] fp16 from the culled half-slabs."""
    smax = plan['smax']
    outs = []
    cf = np.asarray(cam_feats, np.float32)[0]   # [N,D,FH,FW,C]
    for core in plan['cores']:
        fbuf = np.zeros((smax, 128, UJ * C), np.float16)
        for u in core['units']:
            if u['n'] >= 0:
                blk = cf[u['n'], u['d'], u['half'] * HHALF:(u['half'] + 1) * HHALF]
                fbuf[u['slot']] = blk.reshape(128, UJ * C)
        outs.append(fbuf)
    return outs


_CACHE = {}


def _build_bass(plan):
    import concourse.bacc as bacc
    import concourse.mybir as mybir
    import concourse.tile as tile
    from concourse.bass import ds as dslice

    smax, W, H, Rx, Ry = plan['smax'], plan['W'], plan['H'], plan['Rx'], plan['Ry']
    SJ = smax * UJ
    rcells = Rx * Ry
    f32, f16, i32 = mybir.dt.float32, mybir.dt.float16, mybir.dt.int32
    AL = mybir.AluOpType

    nc = bacc.Bacc(None, target_bir_lowering=False, num_devices=NCORES)
    feats_t = nc.dram_tensor("feats", [smax, 128, UJ * C], f16, kind="ExternalInput")
    pxt_t = nc.dram_tensor("pxt", [128, UJ], f32, kind="ExternalInput")
    pyt_t = nc.dram_tensor("pyt", [128, SJ], f32, kind="ExternalInput")
    coef_t = nc.dram_tensor("coef", [128, smax * NCOEF], f32, kind="ExternalInput")
    meta_t = nc.dram_tensor("meta", [128, smax * NMETA], f32, kind="ExternalInput")
    rout_t = nc.dram_tensor("region_out", [C, rcells], f32, kind="ExternalOutput")

    # matmul bank slices within the stripe (PSUM bank = 512 f32)
    bank_slices = [(a, min(a + 512, W)) for a in range(0, W, 512)]

    with tile.TileContext(nc) as tc:
        with tc.tile_pool(name="tabs", bufs=1) as tp, \
             tc.tile_pool(name="geo", bufs=1) as gp, \
             tc.tile_pool(name="work", bufs=3) as wp, \
             tc.tile_pool(name="oh", bufs=1) as op_, \
             tc.tile_pool(name="ps", bufs=2, space="PSUM") as pp, \
             tc.tile_pool(name="dram", bufs=1, space="DRAM") as dp:

            pxt = tp.tile([128, UJ], f32); nc.sync.dma_start(pxt[:], pxt_t[:])
            pyt = tp.tile([128, SJ], f32); nc.sync.dma_start(pyt[:], pyt_t[:])
            coef = tp.tile([128, smax * NCOEF], f32); nc.sync.dma_start(coef[:], coef_t[:])
            meta = tp.tile([128, smax * NMETA], f32); nc.sync.dma_start(meta[:], meta_t[:])
            iota32 = gp.tile([128, W], i32)
            nc.gpsimd.iota(iota32[:], pattern=[[1, W]], base=0, channel_multiplier=0)
            iotaf = tp.tile([128, W], f32)
            nc.vector.tensor_copy(out=iotaf[:], in_=iota32[:])
            region = tp.tile([C, rcells], f32)
            nc.vector.memset(region[:], 0.0)
            stag = tp.tile([C, smax * W], f16)
            lidxT = tp.tile([128, SJ], f32)

            def cslice(kidx):
                ap = coef[:].rearrange("p (s k) -> p s k", k=NCOEF)[:, :, kidx:kidx + 1]
                return ap.broadcast_to([128, smax, UJ])

            def mslice(kidx):
                ap = meta[:].rearrange("p (s k) -> p s k", k=NMETA)[:, :, kidx:kidx + 1]
                return ap.broadcast_to([128, smax, UJ])

            def g3(ap):
                return ap.rearrange("p (s j) -> p s j", j=UJ)

            # ---- batched geometry (identical op order to the reference) ----
            tmpa = gp.tile([128, SJ], f32)
            tmpb = gp.tile([128, SJ], f32)
            pxb = pxt[:][:, None, :].broadcast_to([128, smax, UJ])
            p0 = [gp.tile([128, SJ], f32, name=f'p0_{i}', tag=f'p0_{i}') for i in range(3)]
            for kk in range(3):
                nc.vector.tensor_tensor(out=g3(tmpa[:]), in0=pxb, in1=cslice(3 * kk + 0), op=AL.mult)
                nc.vector.tensor_tensor(out=g3(tmpb[:]), in0=g3(pyt[:]), in1=cslice(3 * kk + 1), op=AL.mult)
                nc.vector.tensor_tensor(out=tmpa[:], in0=tmpa[:], in1=tmpb[:], op=AL.add)
                nc.vector.tensor_tensor(out=g3(p0[kk][:]), in0=g3(tmpa[:]), in1=cslice(3 * kk + 2), op=AL.add)
            uu = gp.tile([128, SJ], f32)
            vv = gp.tile([128, SJ], f32)
            nc.vector.tensor_tensor(out=uu[:], in0=p0[0][:], in1=p0[2][:], op=AL.mult)
            nc.vector.tensor_tensor(out=vv[:], in0=p0[1][:], in1=p0[2][:], op=AL.mult)
            g = [gp.tile([128, SJ], f32, name=f'g_{i}', tag=f'g_{i}') for i in range(3)]
            for kk in range(3):
                base = 9 + 4 * kk
                nc.vector.tensor_tensor(out=g3(tmpa[:]), in0=g3(uu[:]), in1=cslice(base + 0), op=AL.mult)
                nc.vector.tensor_tensor(out=g3(tmpb[:]), in0=g3(vv[:]), in1=cslice(base + 1), op=AL.mult)
                nc.vector.tensor_tensor(out=tmpa[:], in0=tmpa[:], in1=tmpb[:], op=AL.add)
                nc.vector.tensor_tensor(out=g3(tmpb[:]), in0=g3(p0[2][:]), in1=cslice(base + 2), op=AL.mult)
                nc.vector.tensor_tensor(out=tmpa[:], in0=tmpa[:], in1=tmpb[:], op=AL.add)
                nc.vector.tensor_tensor(out=g3(g[kk][:]), in0=g3(tmpa[:]), in1=cslice(base + 3), op=AL.add)
            gx, gy, gz = g

            # ---- direct binning: q = (g - coff) / dx ; k = floor(q) ----
            qx = uu; qy = vv; qz = p0[0]        # reuse buffers
            invx = float(np.float32(1.0) / DXV)
            invz = float(np.float32(1.0) / DZV)
            nc.vector.tensor_scalar(out=qx[:], in0=gx[:], scalar1=float(COFFX),
                                    scalar2=invx, op0=AL.subtract, op1=AL.mult)
            nc.vector.tensor_scalar(out=qy[:], in0=gy[:], scalar1=float(COFFX),
                                    scalar2=invx, op0=AL.subtract, op1=AL.mult)
            nc.vector.tensor_scalar(out=qz[:], in0=gz[:], scalar1=float(COFFZ),
                                    scalar2=invz, op0=AL.subtract, op1=AL.mult)
            kxt = p0[1]; kyt = p0[2]
            # round-to-nearest then subtract (q < r) -> floor
            nc.vector.tensor_scalar(out=kxt[:], in0=qx[:], scalar1=float(MAGIC),
                                    scalar2=float(MAGIC), op0=AL.add, op1=AL.subtract)
            nc.vector.tensor_tensor(out=tmpa[:], in0=qx[:], in1=kxt[:], op=AL.is_lt)
            nc.vector.tensor_tensor(out=kxt[:], in0=kxt[:], in1=tmpa[:], op=AL.subtract)
            nc.vector.tensor_scalar(out=kyt[:], in0=qy[:], scalar1=float(MAGIC),
                                    scalar2=float(MAGIC), op0=AL.add, op1=AL.subtract)
            nc.vector.tensor_tensor(out=tmpa[:], in0=qy[:], in1=kyt[:], op=AL.is_lt)
            nc.vector.tensor_tensor(out=kyt[:], in0=kyt[:], in1=tmpa[:], op=AL.subtract)

            # ---- penalties: window containment + z in (-1, 1) ----
            pen = gx      # reuse
            nc.vector.tensor_tensor(out=g3(pen[:]), in0=g3(kxt[:]), in1=mslice(1), op=AL.is_ge)
            nc.vector.tensor_tensor(out=g3(tmpa[:]), in0=g3(kxt[:]), in1=mslice(2), op=AL.is_lt)
            nc.vector.tensor_tensor(out=pen[:], in0=pen[:], in1=tmpa[:], op=AL.mult)
            nc.vector.tensor_tensor(out=g3(tmpa[:]), in0=g3(kyt[:]), in1=mslice(3), op=AL.is_ge)
            nc.vector.tensor_tensor(out=pen[:], in0=pen[:], in1=tmpa[:], op=AL.mult)
            nc.vector.tensor_tensor(out=g3(tmpa[:]), in0=g3(kyt[:]), in1=mslice(4), op=AL.is_lt)
            nc.vector.tensor_tensor(out=pen[:], in0=pen[:], in1=tmpa[:], op=AL.mult)
            nc.vector.tensor_scalar(out=tmpa[:], in0=qz[:], scalar1=-1.0, scalar2=None, op0=AL.is_gt)
            nc.vector.tensor_tensor(out=pen[:], in0=pen[:], in1=tmpa[:], op=AL.mult)
            nc.vector.tensor_scalar(out=tmpa[:], in0=qz[:], scalar1=1.0, scalar2=None, op0=AL.is_lt)
            nc.vector.tensor_tensor(out=pen[:], in0=pen[:], in1=tmpa[:], op=AL.mult)

            # ---- stripe-local index: kyt*Rx + kxt - D0, clamp, apply penalty ----
            lidx = gy     # reuse
            nc.vector.tensor_scalar(out=lidx[:], in0=kyt[:], scalar1=float(Rx),
                                    scalar2=None, op0=AL.mult)
            nc.vector.tensor_tensor(out=lidx[:], in0=lidx[:], in1=kxt[:], op=AL.add)
            nc.vector.tensor_tensor(out=g3(lidx[:]), in0=g3(lidx[:]), in1=mslice(0), op=AL.subtract)
            nc.vector.tensor_scalar(out=lidx[:], in0=lidx[:], scalar1=-1000.0,
                                    scalar2=40000.0, op0=AL.max, op1=AL.min)
            nc.vector.tensor_scalar(out=tmpa[:], in0=pen[:], scalar1=-BIGPEN,
                                    scalar2=BIGPEN, op0=AL.mult, op1=AL.add)
            nc.vector.tensor_tensor(out=lidxT[:], in0=lidx[:], in1=tmpa[:], op=AL.add)

            region2d = region[:].rearrange("p (y x) -> p y x", x=Rx)
            DVE_JS = tuple(range(0, UJ))
            POOL_JS = ()

            # ---- main unit loop ----
            def body(iv):
                cur = wp.tile([128, UJ], f32, tag="cur")
                nc.sync.dma_start(cur[:], lidxT[:, dslice(iv * UJ, UJ)])
                fb = wp.tile([128, UJ * C], f16, tag="fb")
                nc.sync.dma_start(fb[:], feats_t[dslice(iv, 1)].rearrange("o p x -> p (o x)"))
                oh = op_.tile([128, UJ * W], f16, tag="oh")
                for j in DVE_JS:
                    nc.vector.tensor_scalar(out=oh[:, j * W:(j + 1) * W], in0=iotaf[:],
                                            scalar1=cur[:, j:j + 1], scalar2=None,
                                            op0=AL.is_equal)
                for j in POOL_JS:
                    nc.gpsimd.tensor_scalar(out=oh[:, j * W:(j + 1) * W], in0=iotaf[:],
                                            scalar1=cur[:, j:j + 1], scalar2=None,
                                            op0=AL.is_equal)
                ps = pp.tile([C, W], f32, space="PSUM", tag="ps")
                for j in range(UJ):
                    for (a, bnd) in bank_slices:
                        nc.tensor.matmul(ps[:, a:bnd], lhsT=fb[:, j * C:(j + 1) * C],
                                         rhs=oh[:, j * W + a:j * W + bnd],
                                         start=(j == 0), stop=(j == UJ - 1))
                pss = wp.tile([C, W], f16, tag="pss")
                nc.scalar.copy(out=pss[:], in_=ps[:])
                nc.sync.dma_start(stag[:, dslice(iv * W, W)], pss[:])

            tc.For_i_unrolled(0, smax, 1, body, max_unroll=3)

            # ---- paste: per-class strided reduce over contiguous slots ----
            off = 0
            for ci, (so, cnt) in enumerate(zip(plan['classes'], plan['cls_cnt'])):
                dst = region2d[:, so:so + H, :]
                if cnt == 1:
                    src = stag[:, off * W:(off + 1) * W].rearrange("p (y x) -> p y x", x=Rx)
                    nc.vector.tensor_tensor(out=dst, in0=dst, in1=src, op=AL.add)
                else:
                    src3 = stag[:, off * W:(off + cnt) * W].rearrange(
                        "p (s w) -> p w s", w=W)
                    red = gp.tile([C, W], f32, name=f"red{ci}", tag=f"red{ci}")
                    nc.vector.tensor_reduce(out=red[:], in_=src3, axis=mybir.AxisListType.X,
                                            op=AL.add)
                    nc.vector.tensor_tensor(
                        out=dst, in0=dst,
                        in1=red[:].rearrange("p (y x) -> p y x", x=Rx), op=AL.add)
                off += cnt

            # ---- epilogue: allreduce the region ----
            rpart = dp.tile([C, rcells], f32)
            rsum = dp.tile([C, rcells], f32)
            nc.sync.dma_start(rpart[:], region[:])
            nc.gpsimd.collective_compute(
                "AllReduce", AL.add,
                replica_groups=[list(range(NCORES))],
                ins=[rpart[:]], outs=[rsum[:]])
            nc.sync.dma_start(rout_t[:], rsum[:])

    nc.compile()
    return nc


def _expected_channel_sums(plan, feats):
    """Host replica of the device binning -> per-channel sums of kept feats.

    The scatter only rearranges values, so sum_cells(region[c]) must equal
    sum_kept(feats[c]).  Used as a cheap integrity check on device output.
    """
    f32 = np.float32
    Rx = plan['Rx']
    W = plan['W']
    invx = f32(f32(1.0) / DXV)
    invz = f32(f32(1.0) / DZV)
    tot = np.zeros(C, np.float64)
    for k in range(NCORES):
        cpl = plan['cores'][k]
        smax = plan['smax']
        coef = cpl['coef'][0].reshape(smax, NCOEF)
        meta = cpl['meta'][0].reshape(smax, NMETA)
        px = np.broadcast_to(plan['pxt'][:, None, :], (128, smax, UJ)).astype(f32)
        py = cpl['pyt'].reshape(128, smax, UJ).astype(f32)
        cs = lambda i: coef[None, :, i:i + 1].astype(f32)
        p0 = []
        for kk in range(3):
            ta = f32(px * cs(3 * kk + 0)); tb = f32(py * cs(3 * kk + 1))
            p0.append(f32(f32(ta + tb) + cs(3 * kk + 2)))
        uu = f32(p0[0] * p0[2]); vv = f32(p0[1] * p0[2])
        g = []
        for kk in range(3):
            base = 9 + 4 * kk
            ta = f32(uu * cs(base + 0)); tb = f32(vv * cs(base + 1))
            ta = f32(ta + tb); tb = f32(p0[2] * cs(base + 2))
            g.append(f32(f32(ta + tb) + cs(base + 3)))
        gx, gy, gz = g
        qx = f32(f32(gx - COFFX) * invx)
        qy = f32(f32(gy - COFFX) * invx)
        qz = f32(f32(gz - COFFZ) * invz)

        def floorq(q):
            r = f32(f32(q + MAGIC) - MAGIC)
            return f32(r - (q < r).astype(f32))
        kxt = floorq(qx); kyt = floorq(qy)
        pen = ((kxt >= meta[None, :, 1:2]) & (kxt < meta[None, :, 2:3]) &
               (kyt >= meta[None, :, 3:4]) & (kyt < meta[None, :, 4:5]) &
               (qz > f32(-1.0)) & (qz < f32(1.0)))
        fb = feats[k].reshape(smax, 128, UJ, C).astype(np.float32)
        m = pen.transpose(1, 0, 2)            # [smax, 128, UJ]
        tot += np.einsum('spj,spjc->c', m.astype(np.float64), fb)
    return tot


def kernel(**inputs) -> np.ndarray:
    from concourse.bass_utils import run_bass_kernel_spmd

    plan = _build_plan(inputs)
    key = (plan['smax'], plan['W'], plan['H'], plan['Rx'], plan['Ry'],
           tuple(plan['classes']), tuple(plan['cls_cnt']))
    if key not in _CACHE:
        _CACHE.clear()
        _CACHE[key] = _build_bass(plan)

    feats = _pack_feats(inputs['cam_feats'], plan)
    in_maps = []
    for k in range(NCORES):
        cpl = plan['cores'][k]
        in_maps.append(dict(feats=feats[k], pxt=plan['pxt'], pyt=cpl['pyt'],
                            coef=cpl['coef'], meta=cpl['meta']))
    want = _expected_channel_sums(plan, feats)
    wnorm = np.linalg.norm(want) + 1.0

    region = None
    last_exc = None
    for attempt in range(5):
        try:
            r = run_bass_kernel_spmd(_CACHE[key], in_maps, core_ids=list(range(NCORES)))
            cand = r.results[0]['region_out']    # [C, rcells] summed over cores
            got = cand.astype(np.float64).sum(axis=1)
            region = cand
            if np.linalg.norm(got - want) / wnorm < 0.02:
                break
            print(f"kernel: integrity check failed (attempt {attempt}); retrying",
                  flush=True)
        except Exception as e:
            last_exc = e
            print(f"kernel: execute failed ({type(e).__name__}); retrying", flush=True)
            import time as _time
            _time.sleep(5.0)
        if attempt >= 1:
            _CACHE.clear()
            _CACHE[key] = _build_bass(plan)
    if region is None:
        raise last_exc
    out = np.zeros((B, C, NX, NY), np.float32)
    Rx, Ry = plan['Rx'], plan['Ry']
    blk = region.reshape(C, Ry, Rx).transpose(0, 2, 1)
    out[0, :, plan['rx0']:plan['rx0'] + Rx, plan['ry0']:plan['ry0'] + Ry] = blk
    return out
